# revision 4
# baseline (speedup 1.0000x reference)
"""Trainium2 Bass kernel for nn_BertLayer_22393959481720 (segment_reduce).

Reference computation (per batch item b of 32):
  out_vectors[b, ms, :] = max_j transformer_out[b, segments[b, ms, j], :]   (masks all-ones)
  u = tanh(X @ W_attn + b_attn); logits = u @ w_ctx; a = softmax(logits, axis=S)
  sent_repr[b, :] = sum_s a[s] * X[s, :]

Sharding: pure data parallel, 4 batch items per NeuronCore across 8 cores.

Per-core on-device plan (batch items b = 0..3, X = x[b*512:(b+1)*512, :]):
  - gather: one SWDGE dma_gather per batch pulls the 1024 segment rows from
    HBM into SBUF laid out [ms_partition, j, h] (host pre-orders indices as
    i = j*128 + ms, global row b*512 + s, int16), then one DVE max-reduce
    over the strided innermost j axis -> [128, 768] segment max.
  - attention: PE transposes X -> Xt (h on partitions), matmuls
    U^T = W^T X^T accumulated over 6 k-tiles, tanh(+bias) on ACT,
    logits via 24 N=1 matmuls into [s_partition, s_block] layout,
    unshifted exp on ACT, and sent = a^T @ [X | 1] with the softmax
    denominator folded in as a ones-column; final scale by 1/denom on ACT.
"""

import os
import numpy as np

import concourse.bass as bass
import concourse.bacc as bacc
import concourse.mybir as mybir
import concourse.tile as tile
from concourse.bass_utils import run_bass_kernel_spmd
from concourse.masks import make_identity

P = 128          # partitions
NB = 4           # batch items per core
S = 512          # sequence length
H = 768          # hidden
KT = H // P      # 6 h-tiles
ST = S // P      # 4 s-tiles
MS = 128         # max segments
SL = 8           # segment length
NCORES = 8
NEG_INF = -1e30

F32 = mybir.dt.float32
I16 = mybir.dt.int16


def _emit_batch(nc, b, x, xv, w_sb, bias_sb, wctx_mm, idx_sb, ident,
                outv_d, sent_d, mm_dtype, pools):
    px, pxt, put, pg, psm, psmall, ppt, ppu, ppl, pps = pools
    Tanh = mybir.ActivationFunctionType.Tanh
    Exp = mybir.ActivationFunctionType.Exp

    # ---- load X (with a trailing ones-column for the softmax denom) ----
    xn = px.tile([P, ST, H + 1], F32, tag="xn", name=f"xn{b}")
    nc.sync.dma_start(out=xn[:, :, :H], in_=xv[:, b * ST:(b + 1) * ST, :])
    nc.gpsimd.memset(xn[:, :, H:], 1.0)

    # ---- segment gather + max ----
    g = pg.tile([P, SL, H], F32, tag="g", name=f"g{b}")
    nc.gpsimd.dma_gather(
        out_ap=g[:],
        in_ap=x,
        idxs_ap=idx_sb[:, b * 64:(b + 1) * 64],
        num_idxs=MS * SL,
        num_idxs_reg=MS * SL,
        elem_size=H,
    )
    segmax = psm.tile([P, H], F32, tag="segmax", name=f"segmax{b}")
    nc.vector.tensor_reduce(
        out=segmax[:],
        in_=g[:].rearrange("p j h -> p h j"),
        axis=mybir.AxisListType.X,
        op=mybir.AluOpType.max,
    )
    nc.sync.dma_start(out=outv_d.ap()[b], in_=segmax[:])

    # ---- Xt = X^T (h on partitions) via PE transposes ----
    xt = pxt.tile([P, KT, S], mm_dtype, tag="xt", name=f"xt{b}")
    for kt in range(KT):
        psum_t = ppt.tile([P, S], F32, space="PSUM", tag="pt", name=f"pt{b}_{kt}")
        for t in range(ST):
            nc.tensor.transpose(
                out=psum_t[:, t * P:(t + 1) * P],
                in_=xn[:, t, kt * P:(kt + 1) * P],
                identity=ident[:],
            )
        nc.scalar.copy(out=xt[:, kt, :], in_=psum_t[:])

    # ---- U^T = W^T X^T; u = tanh(U + b) ----
    ut = put.tile([P, KT, S], mm_dtype, tag="ut", name=f"ut{b}")
    for dt in range(KT):
        psum_u = ppu.tile([P, S], F32, space="PSUM", tag="pu", name=f"pu{b}_{dt}")
        for kt in range(KT):
            nc.tensor.matmul(
                out=psum_u[:],
                lhsT=w_sb[:, kt, dt * P:(dt + 1) * P],
                rhs=xt[:, kt, :],
                start=(kt == 0),
                stop=(kt == KT - 1),
            )
        nc.scalar.activation(
            out=ut[:, dt, :], in_=psum_u[:], func=Tanh,
            bias=bias_sb[:, dt:dt + 1],
        )

    # ---- logits[s] = u[s, :] @ w_ctx, laid out [s_in_block, s_block] ----
    psum_l = ppl.tile([P, ST], F32, space="PSUM", tag="pl", name=f"pl{b}")
    for sblk in range(ST):
        for dt in range(KT):
            nc.tensor.matmul(
                out=psum_l[:, sblk:sblk + 1],
                lhsT=ut[:, dt, sblk * P:(sblk + 1) * P],
                rhs=wctx_mm[:, dt:dt + 1],
                start=(dt == 0),
                stop=(dt == KT - 1),
            )
    a2 = psmall.tile([P, ST], mm_dtype, tag="a2", name=f"a2{b}")
    nc.scalar.activation(out=a2[:], in_=psum_l[:], func=Exp)

    # ---- sent_un = a_un^T @ [X | 1]  (last column gives the denom) ----
    psA = pps.tile([1, 512], F32, space="PSUM", tag="psA", name=f"psA{b}")
    psB = pps.tile([1, H + 1 - 512], F32, space="PSUM", tag="psB", name=f"psB{b}")
    for kt in range(ST):
        nc.tensor.matmul(
            out=psA[:], lhsT=a2[:, kt:kt + 1], rhs=xn[:, kt, 0:512],
            start=(kt == 0), stop=(kt == ST - 1),
        )
    for kt in range(ST):
        nc.tensor.matmul(
            out=psB[:], lhsT=a2[:, kt:kt + 1], rhs=xn[:, kt, 512:H + 1],
            start=(kt == 0), stop=(kt == ST - 1),
        )
    recip = psmall.tile([1, 1], F32, tag="recip", name=f"recip{b}")
    nc.vector.reciprocal(out=recip[:], in_=psB[:, H - 512:H - 512 + 1])
    sent_sb = psmall.tile([1, H], F32, tag="sent_sb", name=f"sent_sb{b}")
    nc.scalar.mul(out=sent_sb[:, :512], in_=psA[:], mul=recip[:])
    nc.scalar.mul(out=sent_sb[:, 512:], in_=psB[:, :H - 512], mul=recip[:])
    nc.sync.dma_start(out=sent_d.ap()[b], in_=sent_sb[:])


def _emit(nc, tc, x_d, w_d, b_d, wc_d, idx_d, outv_d, sent_d, mm_dtype, reps=1):
    """Emit the per-core program under a TileContext."""
    from contextlib import ExitStack

    x = x_d.ap()                                # [NB*S, H] DRAM
    xv = x.rearrange("(t p) h -> p t h", p=P)   # [128, NB*ST, H]
    cast = mm_dtype != F32

    with ExitStack() as ctx:
        pconst = ctx.enter_context(tc.tile_pool(name="pconst", bufs=1))
        px = ctx.enter_context(tc.tile_pool(name="px", bufs=2))
        pxt = ctx.enter_context(tc.tile_pool(name="pxt", bufs=2))
        put = ctx.enter_context(tc.tile_pool(name="put", bufs=2))
        pg = ctx.enter_context(tc.tile_pool(name="pg", bufs=2))
        psm = ctx.enter_context(tc.tile_pool(name="psm", bufs=2))
        psmall = ctx.enter_context(tc.tile_pool(name="psmall", bufs=2))
        ppt = ctx.enter_context(tc.tile_pool(name="ppt", bufs=2, space="PSUM"))
        ppu = ctx.enter_context(tc.tile_pool(name="ppu", bufs=2, space="PSUM"))
        ppl = ctx.enter_context(tc.tile_pool(name="ppl", bufs=2, space="PSUM"))
        pps = ctx.enter_context(tc.tile_pool(name="pps", bufs=1, space="PSUM"))
        pools = (px, pxt, put, pg, psm, psmall, ppt, ppu, ppl, pps)

        # ---- resident constants ----
        w_sb = pconst.tile([P, KT, H], mm_dtype)   # W[h, d], h = kt*128 + p
        wv = w_d.ap().rearrange("(kt p) d -> p kt d", p=P)
        if cast:
            nc.gpsimd.dma_start(out=w_sb[:], in_=wv)
        else:
            nc.sync.dma_start(out=w_sb[:], in_=wv)
        bias_sb = pconst.tile([P, KT], F32)        # b[dt*128 + p]
        nc.sync.dma_start(out=bias_sb[:], in_=b_d.ap().rearrange("(dt p) -> p dt", p=P))
        wctx_sb = pconst.tile([P, KT], F32)        # w_ctx[dt*128 + p]
        nc.sync.dma_start(out=wctx_sb[:], in_=wc_d.ap().rearrange("(dt p) -> p dt", p=P))
        if cast:
            wctx_mm = pconst.tile([P, KT], mm_dtype)
            nc.vector.tensor_copy(wctx_mm[:], wctx_sb[:])
        else:
            wctx_mm = wctx_sb
        idx_sb = pconst.tile([P, NB * (MS * SL // 16)], I16)
        nc.sync.dma_start(out=idx_sb[:], in_=idx_d.ap())
        ident = pconst.tile([P, P], F32)
        make_identity(nc, ident[:])

        def body():
            for b in range(NB):
                _emit_batch(nc, b, x, xv, w_sb, bias_sb, wctx_mm, idx_sb,
                            ident, outv_d, sent_d, mm_dtype, pools)

        if reps == 1:
            body()
        else:
            with tc.For_i(0, reps, 1):
                body()


def build_nc(mm_dtype=F32, reps=1):
    nc = bacc.Bacc("TRN2", target_bir_lowering=False, debug=False, num_devices=NCORES)
    x_d = nc.dram_tensor("x", (NB * S, H), F32, kind="ExternalInput")
    w_d = nc.dram_tensor("w", (H, H), F32, kind="ExternalInput")
    b_d = nc.dram_tensor("bias", (H,), F32, kind="ExternalInput")
    wc_d = nc.dram_tensor("wctx", (H,), F32, kind="ExternalInput")
    idx_d = nc.dram_tensor("segidx", (P, NB * (MS * SL // 16)), I16, kind="ExternalInput")
    outv_d = nc.dram_tensor("outv", (NB, MS, H), F32, kind="ExternalOutput")
    sent_d = nc.dram_tensor("sent", (NB, H), F32, kind="ExternalOutput")
    with tile.TileContext(nc) as tc:
        _emit(nc, tc, x_d, w_d, b_d, wc_d, idx_d, outv_d, sent_d, mm_dtype, reps)
    nc.compile()
    return nc


def make_in_maps(transformer_out, segments, W_attn, b_attn, w_ctx):
    """Shard host-side: 4 batch items per core; precompute gather indices."""
    x = np.ascontiguousarray(np.asarray(transformer_out, dtype=np.float32))
    seg = np.asarray(segments).astype(np.int64)
    w = np.ascontiguousarray(np.asarray(W_attn, dtype=np.float32))
    bb = np.ascontiguousarray(np.asarray(b_attn, dtype=np.float32).reshape(H))
    wc = np.ascontiguousarray(np.asarray(w_ctx, dtype=np.float32).reshape(H))

    in_maps = []
    for c in range(NCORES):
        xs = x[c * NB:(c + 1) * NB].reshape(NB * S, H)
        blocks = []
        for b in range(NB):
            gl = seg[c * NB + b] + b * S          # [MS, SL] global row ids
            arr = gl.T.reshape(MS * SL)           # i = j*128 + ms
            blocks.append(arr.reshape(MS * SL // 16, 16).T)   # [16, 64]
        blk = np.concatenate(blocks, axis=1)      # [16, NB*64]
        segidx = np.tile(blk, (P // 16, 1)).astype(np.int16)  # replicate for 8 Q7 cores
        in_maps.append({
            "x": np.ascontiguousarray(xs),
            "w": w,
            "bias": bb,
            "wctx": wc,
            "segidx": np.ascontiguousarray(segidx),
        })
    return in_maps


_STATE = {}


def _get_nc():
    key = os.environ.get("KERNEL_MM_DTYPE", "fp32")
    if key not in _STATE:
        mm = {"fp32": F32, "f32r": mybir.dt.float32r, "bf16": mybir.dt.bfloat16}[key]
        _STATE[key] = build_nc(mm)
    return _STATE[key]


def kernel(transformer_out, segments, segments_mask, segments_indices_mask,
           W_attn, b_attn, w_ctx):
    transformer_out = np.asarray(transformer_out, dtype=np.float32)
    segments_mask = np.asarray(segments_mask).astype(bool)
    sim = np.asarray(segments_indices_mask).astype(bool)

    # Host-side handling of segments_indices_mask (all-ones in the graded
    # inputs): point masked slots at a valid index of the same segment so the
    # on-device max ignores them; rows with no valid index become NEG_INF.
    seg_eff = np.asarray(segments).astype(np.int64)
    any_valid = sim.any(axis=-1)
    if not sim.all():
        first_valid = np.argmax(sim, axis=-1)
        fv = np.take_along_axis(seg_eff, first_valid[..., None], axis=-1)
        seg_eff = np.where(sim, seg_eff, fv)

    nc = _get_nc()
    in_maps = make_in_maps(transformer_out, seg_eff, W_attn, b_attn, w_ctx)
    res = run_bass_kernel_spmd(nc, in_maps, core_ids=list(range(NCORES)))
    outs = res.results
    out_vectors = np.concatenate([m["outv"] for m in outs], axis=0)
    sent_repr = np.concatenate([m["sent"] for m in outs], axis=0)

    if not sim.all():
        out_vectors = np.where(any_valid[..., None], out_vectors, np.float32(NEG_INF))
    if not segments_mask.all():
        pad_vec = transformer_out[:, -2, :]
        out_vectors = np.where(segments_mask[..., None], out_vectors,
                               pad_vec[:, None, :])

    return out_vectors.astype(np.float32), sent_repr.astype(np.float32)


# revision 5
# speedup vs baseline: 1.9657x; 1.9657x over previous
"""Trainium2 Bass kernel for nn_BertLayer_22393959481720 (segment_reduce).

Reference computation (per batch item b of 32):
  out_vectors[b, ms, :] = max_j transformer_out[b, segments[b, ms, j], :]   (masks all-ones)
  u = tanh(X @ W_attn + b_attn); logits = u @ w_ctx; a = softmax(logits, axis=S)
  sent_repr[b, :] = sum_s a[s] * X[s, :]

Sharding: pure data parallel, 4 batch items per NeuronCore across 8 cores.

Per-core on-device plan (batch items b = 0..3, X = x[b*512:(b+1)*512, :]):
  - gather: one SWDGE dma_gather per batch pulls the 1024 segment rows from
    HBM into SBUF laid out [ms_partition, j, h] (host pre-orders indices as
    i = j*128 + ms, global row b*512 + s, int16), then one DVE max-reduce
    over the strided innermost j axis -> [128, 768] segment max.
  - attention: PE transposes X -> Xt (h on partitions), matmuls
    U^T = W^T X^T accumulated over 6 k-tiles, tanh(+bias) on ACT,
    logits via 24 N=1 matmuls into [s_partition, s_block] layout,
    unshifted exp on ACT, and sent = a^T @ [X | 1] with the softmax
    denominator folded in as a ones-column; final scale by 1/denom on ACT.
"""

import os
import numpy as np

import concourse.bass as bass
import concourse.bacc as bacc
import concourse.mybir as mybir
import concourse.tile as tile
from concourse.bass_utils import run_bass_kernel_spmd
from concourse.masks import make_identity

P = 128          # partitions
NB = 4           # batch items per core
S = 512          # sequence length
H = 768          # hidden
KT = H // P      # 6 h-tiles
ST = S // P      # 4 s-tiles
MS = 128         # max segments
SL = 8           # segment length
NCORES = 8
NEG_INF = -1e30

F32 = mybir.dt.float32
I16 = mybir.dt.int16


def _emit_batch(nc, b, x, xv, w_sb, bias_sb, wctx_mm, idx_sb, ident,
                outv_d, sent_d, mm_dtype, pools):
    px, pxt, put, pg, psm, psmall, ppt, ppu, ppl, pps = pools
    Tanh = mybir.ActivationFunctionType.Tanh
    Exp = mybir.ActivationFunctionType.Exp

    # ---- load X (with a trailing ones-column for the softmax denom) ----
    xn = px.tile([P, ST, H + 1], F32, tag="xn", name=f"xn{b}")
    nc.sync.dma_start(out=xn[:, :, :H], in_=xv[:, b * ST:(b + 1) * ST, :])
    nc.gpsimd.memset(xn[:, :, H:], 1.0)

    # ---- segment gather + max ----
    g = pg.tile([P, SL, H], F32, tag="g", name=f"g{b}")
    nc.gpsimd.dma_gather(
        out_ap=g[:],
        in_ap=x,
        idxs_ap=idx_sb[:, b * 64:(b + 1) * 64],
        num_idxs=MS * SL,
        num_idxs_reg=MS * SL,
        elem_size=H,
    )
    segmax = psm.tile([P, H], F32, tag="segmax", name=f"segmax{b}")
    nc.vector.tensor_reduce(
        out=segmax[:],
        in_=g[:].rearrange("p j h -> p h j"),
        axis=mybir.AxisListType.X,
        op=mybir.AluOpType.max,
    )
    nc.sync.dma_start(out=outv_d.ap()[b], in_=segmax[:])

    # ---- Xt = X^T (h on partitions) via PE transposes ----
    xt = pxt.tile([P, KT, S], mm_dtype, tag="xt", name=f"xt{b}")
    for kt in range(KT):
        psum_t = ppt.tile([P, S], F32, space="PSUM", tag="pt", name=f"pt{b}_{kt}")
        for t in range(ST):
            nc.tensor.transpose(
                out=psum_t[:, t * P:(t + 1) * P],
                in_=xn[:, t, kt * P:(kt + 1) * P],
                identity=ident[:],
            )
        nc.scalar.copy(out=xt[:, kt, :], in_=psum_t[:])

    # ---- U^T = W^T X^T; u = tanh(U + b) ----
    ut = put.tile([P, KT, S], mm_dtype, tag="ut", name=f"ut{b}")
    for dt in range(KT):
        psum_u = ppu.tile([P, S], F32, space="PSUM", tag="pu", name=f"pu{b}_{dt}")
        for kt in range(KT):
            nc.tensor.matmul(
                out=psum_u[:],
                lhsT=w_sb[:, kt, dt * P:(dt + 1) * P],
                rhs=xt[:, kt, :],
                start=(kt == 0),
                stop=(kt == KT - 1),
            )
        nc.scalar.activation(
            out=ut[:, dt, :], in_=psum_u[:], func=Tanh,
            bias=bias_sb[:, dt:dt + 1],
        )

    # ---- logits[s] = u[s, :] @ w_ctx, laid out [s_in_block, s_block] ----
    psum_l = ppl.tile([P, ST], F32, space="PSUM", tag="pl", name=f"pl{b}")
    for sblk in range(ST):
        for dt in range(KT):
            nc.tensor.matmul(
                out=psum_l[:, sblk:sblk + 1],
                lhsT=ut[:, dt, sblk * P:(sblk + 1) * P],
                rhs=wctx_mm[:, dt:dt + 1],
                start=(dt == 0),
                stop=(dt == KT - 1),
            )
    a2 = psmall.tile([P, ST], F32, tag="a2", name=f"a2{b}")
    nc.scalar.activation(out=a2[:], in_=psum_l[:], func=Exp)

    # ---- sent_un = a_un^T @ [X | 1]  (last column gives the denom) ----
    psA = pps.tile([1, 512], F32, space="PSUM", tag="psA", name=f"psA{b}")
    psB = pps.tile([1, H + 1 - 512], F32, space="PSUM", tag="psB", name=f"psB{b}")
    for kt in range(ST):
        nc.tensor.matmul(
            out=psA[:], lhsT=a2[:, kt:kt + 1], rhs=xn[:, kt, 0:512],
            start=(kt == 0), stop=(kt == ST - 1),
        )
    for kt in range(ST):
        nc.tensor.matmul(
            out=psB[:], lhsT=a2[:, kt:kt + 1], rhs=xn[:, kt, 512:H + 1],
            start=(kt == 0), stop=(kt == ST - 1),
        )
    recip = psmall.tile([1, 1], F32, tag="recip", name=f"recip{b}")
    nc.vector.reciprocal(out=recip[:], in_=psB[:, H - 512:H - 512 + 1])
    sent_sb = psmall.tile([1, H], F32, tag="sent_sb", name=f"sent_sb{b}")
    nc.scalar.mul(out=sent_sb[:, :512], in_=psA[:], mul=recip[:])
    nc.scalar.mul(out=sent_sb[:, 512:], in_=psB[:, :H - 512], mul=recip[:])
    nc.sync.dma_start(out=sent_d.ap()[b], in_=sent_sb[:])


def _emit(nc, tc, x_d, w_d, b_d, wc_d, idx_d, outv_d, sent_d, mm_dtype, reps=1):
    """Emit the per-core program under a TileContext."""
    from contextlib import ExitStack

    x = x_d.ap()                                # [NB*S, H] DRAM
    xv = x.rearrange("(t p) h -> p t h", p=P)   # [128, NB*ST, H]
    cast = mm_dtype != F32

    with ExitStack() as ctx:
        pconst = ctx.enter_context(tc.tile_pool(name="pconst", bufs=1))
        px = ctx.enter_context(tc.tile_pool(name="px", bufs=2))
        pxt = ctx.enter_context(tc.tile_pool(name="pxt", bufs=2))
        put = ctx.enter_context(tc.tile_pool(name="put", bufs=2))
        pg = ctx.enter_context(tc.tile_pool(name="pg", bufs=2))
        psm = ctx.enter_context(tc.tile_pool(name="psm", bufs=2))
        psmall = ctx.enter_context(tc.tile_pool(name="psmall", bufs=2))
        ppt = ctx.enter_context(tc.tile_pool(name="ppt", bufs=2, space="PSUM"))
        ppu = ctx.enter_context(tc.tile_pool(name="ppu", bufs=2, space="PSUM"))
        ppl = ctx.enter_context(tc.tile_pool(name="ppl", bufs=2, space="PSUM"))
        pps = ctx.enter_context(tc.tile_pool(name="pps", bufs=1, space="PSUM"))
        pools = (px, pxt, put, pg, psm, psmall, ppt, ppu, ppl, pps)

        # ---- resident constants ----
        w_sb = pconst.tile([P, KT, H], mm_dtype)   # W[h, d], h = kt*128 + p
        wv = w_d.ap().rearrange("(kt p) d -> p kt d", p=P)
        if cast:
            nc.gpsimd.dma_start(out=w_sb[:], in_=wv)
        else:
            nc.sync.dma_start(out=w_sb[:], in_=wv)
        bias_sb = pconst.tile([P, KT], F32)        # b[dt*128 + p]
        nc.sync.dma_start(out=bias_sb[:], in_=b_d.ap().rearrange("(dt p) -> p dt", p=P))
        wctx_sb = pconst.tile([P, KT], F32)        # w_ctx[dt*128 + p]
        nc.sync.dma_start(out=wctx_sb[:], in_=wc_d.ap().rearrange("(dt p) -> p dt", p=P))
        if cast:
            wctx_mm = pconst.tile([P, KT], mm_dtype)
            nc.vector.tensor_copy(wctx_mm[:], wctx_sb[:])
        else:
            wctx_mm = wctx_sb
        idx_sb = pconst.tile([P, NB * (MS * SL // 16)], I16)
        nc.sync.dma_start(out=idx_sb[:], in_=idx_d.ap())
        ident = pconst.tile([P, P], F32)
        make_identity(nc, ident[:])

        def body():
            for b in range(NB):
                _emit_batch(nc, b, x, xv, w_sb, bias_sb, wctx_mm, idx_sb,
                            ident, outv_d, sent_d, mm_dtype, pools)

        if reps == 1:
            body()
        else:
            with tc.For_i(0, reps, 1):
                body()


def build_nc(mm_dtype=F32, reps=1):
    nc = bacc.Bacc("TRN2", target_bir_lowering=False, debug=False, num_devices=NCORES)
    x_d = nc.dram_tensor("x", (NB * S, H), F32, kind="ExternalInput")
    w_d = nc.dram_tensor("w", (H, H), F32, kind="ExternalInput")
    b_d = nc.dram_tensor("bias", (H,), F32, kind="ExternalInput")
    wc_d = nc.dram_tensor("wctx", (H,), F32, kind="ExternalInput")
    idx_d = nc.dram_tensor("segidx", (P, NB * (MS * SL // 16)), I16, kind="ExternalInput")
    outv_d = nc.dram_tensor("outv", (NB, MS, H), F32, kind="ExternalOutput")
    sent_d = nc.dram_tensor("sent", (NB, H), F32, kind="ExternalOutput")
    with tile.TileContext(nc) as tc:
        _emit(nc, tc, x_d, w_d, b_d, wc_d, idx_d, outv_d, sent_d, mm_dtype, reps)
    nc.compile()
    return nc


def make_in_maps(transformer_out, segments, W_attn, b_attn, w_ctx):
    """Shard host-side: 4 batch items per core; precompute gather indices."""
    x = np.ascontiguousarray(np.asarray(transformer_out, dtype=np.float32))
    seg = np.asarray(segments).astype(np.int64)
    w = np.ascontiguousarray(np.asarray(W_attn, dtype=np.float32))
    bb = np.ascontiguousarray(np.asarray(b_attn, dtype=np.float32).reshape(H))
    wc = np.ascontiguousarray(np.asarray(w_ctx, dtype=np.float32).reshape(H))

    in_maps = []
    for c in range(NCORES):
        xs = x[c * NB:(c + 1) * NB].reshape(NB * S, H)
        blocks = []
        for b in range(NB):
            gl = seg[c * NB + b] + b * S          # [MS, SL] global row ids
            arr = gl.T.reshape(MS * SL)           # i = j*128 + ms
            blocks.append(arr.reshape(MS * SL // 16, 16).T)   # [16, 64]
        blk = np.concatenate(blocks, axis=1)      # [16, NB*64]
        segidx = np.tile(blk, (P // 16, 1)).astype(np.int16)  # replicate for 8 Q7 cores
        in_maps.append({
            "x": np.ascontiguousarray(xs),
            "w": w,
            "bias": bb,
            "wctx": wc,
            "segidx": np.ascontiguousarray(segidx),
        })
    return in_maps


_STATE = {}


def _get_nc():
    key = os.environ.get("KERNEL_MM_DTYPE", "fp32")
    if key not in _STATE:
        mm = {"fp32": F32, "f32r": mybir.dt.float32r, "bf16": mybir.dt.bfloat16}[key]
        _STATE[key] = build_nc(mm)
    return _STATE[key]


def kernel(transformer_out, segments, segments_mask, segments_indices_mask,
           W_attn, b_attn, w_ctx):
    transformer_out = np.asarray(transformer_out, dtype=np.float32)
    segments_mask = np.asarray(segments_mask).astype(bool)
    sim = np.asarray(segments_indices_mask).astype(bool)

    # Host-side handling of segments_indices_mask (all-ones in the graded
    # inputs): point masked slots at a valid index of the same segment so the
    # on-device max ignores them; rows with no valid index become NEG_INF.
    seg_eff = np.asarray(segments).astype(np.int64)
    any_valid = sim.any(axis=-1)
    if not sim.all():
        first_valid = np.argmax(sim, axis=-1)
        fv = np.take_along_axis(seg_eff, first_valid[..., None], axis=-1)
        seg_eff = np.where(sim, seg_eff, fv)

    nc = _get_nc()
    in_maps = make_in_maps(transformer_out, seg_eff, W_attn, b_attn, w_ctx)
    res = run_bass_kernel_spmd(nc, in_maps, core_ids=list(range(NCORES)))
    outs = res.results
    out_vectors = np.concatenate([m["outv"] for m in outs], axis=0)
    sent_repr = np.concatenate([m["sent"] for m in outs], axis=0)

    if not sim.all():
        out_vectors = np.where(any_valid[..., None], out_vectors, np.float32(NEG_INF))
    if not segments_mask.all():
        pad_vec = transformer_out[:, -2, :]
        out_vectors = np.where(segments_mask[..., None], out_vectors,
                               pad_vec[:, None, :])

    return out_vectors.astype(np.float32), sent_repr.astype(np.float32)
